# revision 8
# baseline (speedup 1.0000x reference)
"""BlockSparseLinear on 8 TRN2 NeuronCores — block-sparse PE-tiled kernel (v3).

Computes out = x @ W_dense.T + bias where W_dense is [4096, 4096] assembled
from 8192 nonzero 32x32 blocks (50% density).

Strategy:
  - Pure 8-way token sharding: each core gets 512 tokens, all 8192 blocks.
    The sparsity pattern is shared, so all cores run one SPMD program.
  - Only nonzero blocks are computed, via PE sub-array tiling.  The v2
    lesson: every matmul pays ~34ns on the serial weight-load path
    (LDWEIGHTS streams its 32 columns at 1.2 GHz) regardless of stationary
    height.  So blocks are packed into the TALLEST stationaries possible:
    an x "slot" m holds 4 k-blocks (bands i=0..3 at partitions 32i); a row
    with >=2 blocks in one slot gets a single [128k, 32o] stationary
    (QUAD, absent bands zeroed, one LDW for up to 4 blocks).  Lone blocks
    stay [32k, 32o] SINGLEs.  A global knob balances total instruction
    issue (~34ns each) against PE stream occupancy (~15ns per block-slot).
  - Rows are processed in 32 windows of 4 rows (one per column strip j).
    PSUM discipline: tile -> bank 4p + lowest-band (p = window parity);
    same-(bank,slice) writers always share a sub-array path so hardware
    FIFO serializes them; different slices of one bank may be written
    concurrently (v2-proven).  Dummy zero-weight singles cover (row, band)
    groups that would otherwise leave a psum slice unstarted.
  - Per window, quads are emitted first, then singles (fewer PE tiling
    mode switches), each round-robin across strips/bands.
  - DVE combines the 4 partial banks + bias per window -> DMA out.
"""

import os
from itertools import permutations

import numpy as np
from ml_dtypes import bfloat16

import concourse.mybir as mybir
import concourse.tile as tile
from concourse import bacc
from concourse.bass_utils import run_bass_kernel_spmd

BLOCK = 32
IN_FEATURES = 4096
OUT_FEATURES = 4096
N_TOKENS = 4096
IB = IN_FEATURES // BLOCK  # 128 block-cols
OB = OUT_FEATURES // BLOCK  # 128 block-rows

N_CORES = 8
TSH = N_TOKENS // N_CORES  # 512 tokens per core
NFREE = 512
P = 128

NWIN = 32  # windows of 4 rows
CHUNK_SLOTS = 64  # 32-col weight slots per DMA chunk
X_CHUNKS = 8

ISSUE_NS = 34.0  # measured per-instruction issue cost (LDW+MM pair)
UNIT_NS = 15.0  # per block-slot stream cost (512 cyc / 16 sub-arrays)

LAST_EXEC_NS = None
LAST_RESULT = None


def _install_axon_ntff_hook():
    try:
        from antenv.axon_hooks import get_axon_ntff_profile_hook

        return get_axon_ntff_profile_hook() is not None
    except ImportError:
        pass
    try:
        import sys
        import types

        import antenv
        import trn_agent_boot.trn_boot as tb

        hook = tb._ntff_profile_via_ctypes("/opt/axon/libaxon_pjrt.so")
        if hook is None:
            return False
        mod = types.ModuleType("antenv.axon_hooks")
        mod._hook = hook
        mod.get_axon_ntff_profile_hook = lambda: mod._hook
        mod.set_axon_ntff_profile_hook = lambda h: setattr(mod, "_hook", h)
        sys.modules["antenv.axon_hooks"] = mod
        antenv.axon_hooks = mod

        import concourse.bass_utils as bu

        bu.upload_artifacts = lambda tmpdir: str(tmpdir)
        return True
    except Exception:
        return False


class _Tile:
    """One PE instruction: a QUAD ([128,32] stationary) or SINGLE ([32,32])."""

    __slots__ = ("kind", "row", "m", "blocks", "j", "w", "slot", "start", "stop")

    def __init__(self, kind, row, m, blocks):
        self.kind = kind  # 'q' or 's'
        self.row = row
        self.m = m
        self.blocks = blocks  # list of (band, bidx); bidx -1 = zero dummy
        self.j = -1
        self.w = -1
        self.slot = -1
        self.start = False
        self.stop = False

    @property
    def lowband(self):
        return 0 if self.kind == "q" else self.blocks[0][0]


def _schedule(block_ids):
    ids = np.asarray(block_ids, dtype=np.int64)
    r_all = ids // IB
    c_all = ids % IB

    # per-row: dict m -> list of (band, bidx)
    row_slots = [dict() for _ in range(OB)]
    for b in range(len(ids)):
        r = int(r_all[b])
        c = int(c_all[b])
        row_slots[r].setdefault(c // 4, []).append((c % 4, b))

    # Phase-serial cost model (measured): quad = 53.3ns flat (stream-bound,
    # 4-way strip concurrency), single = 29.5ns flat (LDW-issue-bound).
    # k>=2 slots always win as quads; k==1 stays a single.
    row_tiles = []
    for r in range(OB):
        tiles = []
        for m in sorted(row_slots[r]):
            blks = sorted(row_slots[r][m])
            if len(blks) == 1:
                for band, bidx in blks:
                    tiles.append(_Tile("s", r, m, [(band, bidx)]))
            else:
                tiles.append(_Tile("q", r, m, blks))
        # guarantee every band group has a writer (psum slice validity)
        havebands = set(t.lowband for t in tiles)
        if not any(t.kind == "q" for t in tiles) and 0 not in havebands:
            pass  # band 0 covered below by dummy single if needed
        for b in range(4):
            if b not in havebands:
                tiles.append(_Tile("s", r, 0, [(b, -1)]))
        row_tiles.append(tiles)

    units = np.array(
        [sum(4 if t.kind == "q" else 1 for t in tiles) for tiles in row_tiles]
    )

    # windows: snake by units, then per-window strip assignment balancing
    order = np.argsort(-units, kind="stable")
    win_rows = [order[4 * w : 4 * w + 4] for w in range(NWIN)]
    tot_u = np.zeros(4)
    tot_q = np.zeros(4)
    assign = np.zeros((NWIN, 4), dtype=np.int64)  # [w, j] -> row
    for w in range(NWIN):
        rows4 = win_rows[w]
        u4 = units[rows4].astype(float)
        q4 = np.array(
            [sum(1 for t in row_tiles[r] if t.kind == "q") for r in rows4],
            dtype=float,
        )
        best = None
        for perm in permutations(range(4)):
            tu = tot_u.copy()
            tq = tot_q.copy()
            for k in range(4):
                tu[perm[k]] += u4[k]
                tq[perm[k]] += q4[k]
            score = (tq.max() - tq.min(), tu.max() - tu.min())
            if best is None or score < best[0]:
                best = (score, perm)
        perm = best[1]
        for k in range(4):
            j = perm[k]
            r = int(rows4[k])
            assign[w, j] = r
            for t in row_tiles[r]:
                t.j = j
                t.w = w
            tot_u[j] += u4[k]
            tot_q[j] += q4[k]

    # emission order + slot assignment; windows are processed in PAIRS so
    # each quad->single tiling-mode switch covers two windows (8 psum banks)
    emit = []  # list over window-pairs of instruction lists
    slot_base = 0
    for wp in range(NWIN // 2):
        wl = []
        qlists = []
        slists = []
        for w in (2 * wp, 2 * wp + 1):
            for j in range(4):
                r = int(assign[w, j])
                strips = row_tiles[r]
                qlists.append([t for t in strips if t.kind == "q"])
                ss = [t for t in strips if t.kind == "s"]
                # band-interleave this strip's singles
                byband = [[], [], [], []]
                for t in ss:
                    byband[t.lowband].append(t)
                inter = []
                k = 0
                while any(byband):
                    b = k % 4
                    if byband[b]:
                        inter.append(byband[b].pop(0))
                    k += 1
                slists.append(inter)
        qi = [0] * 8
        while True:
            prog = False
            for j in range(8):
                if qi[j] < len(qlists[j]):
                    t = qlists[j][qi[j]]
                    t.slot = slot_base
                    slot_base += 1
                    wl.append(t)
                    qi[j] += 1
                    prog = True
            if not prog:
                break
        lane = [0, 0, 0, 0]  # per-band lane counter for singles slots
        si = [0] * 8
        sl = []
        while True:
            prog = False
            for j in range(8):
                if si[j] < len(slists[j]):
                    t = slists[j][si[j]]
                    b = t.lowband
                    t.slot = slot_base + lane[b]
                    lane[b] += 1
                    sl.append(t)
                    si[j] += 1
                    prog = True
            if not prog:
                break
        slot_base += max(lane) if any(lane) else 0
        wl.extend(sl)
        emit.append(wl)

    # start/stop flags per (row, lowband) group, in emission order
    groups = {}
    for wl in emit:
        for t in wl:
            groups.setdefault((t.row, t.lowband), []).append(t)
    for key, ts in groups.items():
        ts[0].start = True
        ts[-1].stop = True

    n_slots = slot_base
    nch = (n_slots + CHUNK_SLOTS - 1) // CHUNK_SLOTS
    return {
        "emit": emit,
        "assign": assign,
        "NCH": nch,
        "n_slots": n_slots,
        "n_instr": sum(len(wl) for wl in emit),
    }


def _build_bass(sched):
    nch = sched["NCH"]
    emit = sched["emit"]

    nc = bacc.Bacc(None, target_bir_lowering=False)

    x_d = nc.dram_tensor("xh", [P, 32, TSH], mybir.dt.bfloat16, kind="ExternalInput")
    w_d = nc.dram_tensor(
        "wst", [nch, P, CHUNK_SLOTS * BLOCK], mybir.dt.bfloat16, kind="ExternalInput"
    )
    b_d = nc.dram_tensor("biasq", [P, NWIN], mybir.dt.float32, kind="ExternalInput")
    o_d = nc.dram_tensor("out", [NWIN, P, TSH], mybir.dt.float32, kind="ExternalOutput")

    with tile.TileContext(nc) as tc:
        with (
            tc.tile_pool(name="xpool", bufs=1) as xpool,
            tc.tile_pool(name="wpool", bufs=4) as wpool,
            tc.tile_pool(name="spool", bufs=2) as spool,
            tc.tile_pool(name="bpool", bufs=1) as bpool,
            tc.tile_pool(name="psum", bufs=2, space="PSUM") as ppool,
        ):
            bias_sb = bpool.tile([P, NWIN], mybir.dt.float32)
            nc.scalar.dma_start(bias_sb[:], b_d[:])

            # x in per-chunk tiles so matmul deps are chunk-granular (the
            # first matmuls must not wait for the whole 4.2MB of x)
            MCH = 32 // X_CHUNKS
            x_tiles = []
            for xc in range(X_CHUNKS):
                xt = xpool.tile(
                    [P, MCH, TSH], mybir.dt.bfloat16, tag=f"x{xc}", name="x"
                )
                x_tiles.append(xt)

            w_tiles = {}
            W_PREFETCH = 4

            def issue_w(ch):
                if ch < nch and ch not in w_tiles:
                    w_sb = wpool.tile(
                        [P, CHUNK_SLOTS * BLOCK], mybir.dt.bfloat16, tag="w", name="w"
                    )
                    nc.sync.dma_start(w_sb[:], w_d[ch])
                    w_tiles[ch] = w_sb

            # interleave the x and early-w DMAs across the two rings:
            # scalar: x0, x2, x4, x6  |  sync: w0, x1, w1, x3, w2, x5, w3, x7
            nc.scalar.dma_start(x_tiles[0][:], x_d[:, 0:MCH, :])
            issue_w(0)
            for k, xc in enumerate((1, 3, 5, 7)):
                issue_w(k + 1)
                nc.sync.dma_start(
                    x_tiles[xc][:], x_d[:, xc * MCH : (xc + 1) * MCH, :]
                )
            for xc in (2, 4, 6):
                nc.scalar.dma_start(
                    x_tiles[xc][:], x_d[:, xc * MCH : (xc + 1) * MCH, :]
                )

            ps_tiles = {}
            win_left = [16] * NWIN
            issued_ch = 0

            for wl in emit:
                for t in wl:
                    ch = t.slot // CHUNK_SLOTS
                    while issued_ch < ch:
                        issued_ch += 1
                        issue_w(issued_ch + W_PREFETCH)
                    w_sb = w_tiles[ch]
                    col0 = (t.slot % CHUNK_SLOTS) * BLOCK
                    key = (t.w, t.lowband)
                    if key not in ps_tiles:
                        ps_tiles[key] = ppool.tile(
                            [P, NFREE],
                            mybir.dt.float32,
                            tag=f"ps{t.lowband}",
                            name="ps",
                        )
                    psum_t = ps_tiles[key]
                    j = t.j
                    x_sb = x_tiles[t.m // MCH]
                    mloc = t.m % MCH
                    if t.kind == "q":
                        nc.tensor.matmul(
                            psum_t[32 * j : 32 * j + 32, :],
                            lhsT=w_sb[:, col0 : col0 + BLOCK],
                            rhs=x_sb[:, mloc, :],
                            start=t.start,
                            stop=t.stop,
                            tile_position=(0, 32 * j),
                        )
                    else:
                        b = t.lowband
                        nc.tensor.matmul(
                            psum_t[32 * j : 32 * j + 32, :],
                            lhsT=w_sb[32 * b : 32 * b + 32, col0 : col0 + BLOCK],
                            rhs=x_sb[32 * b : 32 * b + 32, mloc, :],
                            start=t.start,
                            stop=t.stop,
                            tile_position=(32 * b, 32 * j),
                        )
                    if t.stop:
                        win_left[t.w] -= 1
                        if win_left[t.w] == 0:
                            q = t.w
                            pt = [ps_tiles.pop((q, ii)) for ii in range(4)]
                            s1 = spool.tile([P, NFREE], mybir.dt.float32, tag="s1")
                            nc.vector.tensor_tensor(
                                s1[:],
                                pt[0][:],
                                bias_sb[:, q : q + 1].to_broadcast([P, NFREE]),
                                mybir.AluOpType.add,
                            )
                            s2 = spool.tile([P, NFREE], mybir.dt.float32, tag="s2")
                            nc.vector.tensor_tensor(
                                s2[:], pt[1][:], s1[:], mybir.AluOpType.add
                            )
                            s3 = spool.tile([P, NFREE], mybir.dt.float32, tag="s3")
                            nc.vector.tensor_tensor(
                                s3[:], pt[2][:], s2[:], mybir.AluOpType.add
                            )
                            so = spool.tile([P, NFREE], mybir.dt.float32, tag="so")
                            nc.vector.tensor_tensor(
                                so[:], pt[3][:], s3[:], mybir.AluOpType.add
                            )
                            nc.scalar.dma_start(o_d[q], so[:])

    nc.compile()
    return nc


def _prep_weights(weight_data, sched):
    nch = sched["NCH"]
    wdT = np.ascontiguousarray(weight_data.transpose(0, 2, 1)).astype(bfloat16)
    w_np = np.zeros((nch, P, CHUNK_SLOTS * BLOCK), dtype=bfloat16)
    for wl in sched["emit"]:
        for t in wl:
            ch = t.slot // CHUNK_SLOTS
            col0 = (t.slot % CHUNK_SLOTS) * BLOCK
            for band, bidx in t.blocks:
                if bidx >= 0:
                    w_np[ch, 32 * band : 32 * band + 32, col0 : col0 + BLOCK] = wdT[
                        bidx
                    ]
    return w_np


def _prep_x(x_shard):
    a = np.ascontiguousarray(x_shard.T).reshape(IB, BLOCK, TSH)  # [c, q, t]
    b = a.reshape(32, 4, BLOCK, TSH).transpose(1, 2, 0, 3)  # [i, q, m, t]
    return np.ascontiguousarray(b.reshape(P, 32, TSH)).astype(bfloat16)


def _prep_bias(bias, sched):
    assign = sched["assign"]
    bias_np = np.zeros((P, NWIN), dtype=np.float32)
    for q in range(NWIN):
        for j in range(4):
            r = int(assign[q, j])
            bias_np[32 * j : 32 * j + 32, q] = bias[32 * r : 32 * r + 32]
    return bias_np


def _assemble_out(o_cores, sched):
    assign = sched["assign"]
    rflat = assign.reshape(-1)
    out = np.empty((N_TOKENS, OUT_FEATURES), dtype=np.float32)
    for core, o in enumerate(o_cores):
        o4 = o.reshape(NWIN, 4, BLOCK, TSH)
        flat = o4.transpose(3, 0, 1, 2).reshape(TSH, OB, BLOCK)
        view = out[core * TSH : (core + 1) * TSH].reshape(TSH, OB, BLOCK)
        view[:, rflat, :] = flat
    return out


def _emulate_core(xh, w_np, bias_np, sched):
    o_d = np.zeros((NWIN, P, TSH), dtype=np.float32)
    psum = {}
    for wl in sched["emit"]:
        for t in wl:
            key = (t.w, t.lowband)
            if key not in psum:
                psum[key] = np.zeros((P, NFREE), dtype=np.float32)
            if t.start:
                psum[key][32 * t.j : 32 * t.j + 32, :] = 0.0
            ch = t.slot // CHUNK_SLOTS
            col0 = (t.slot % CHUNK_SLOTS) * BLOCK
            if t.kind == "q":
                lhsT = w_np[ch, :, col0 : col0 + BLOCK].astype(np.float32)
                rhs = xh[:, t.m, :].astype(np.float32)
            else:
                b = t.lowband
                lhsT = w_np[ch, 32 * b : 32 * b + 32, col0 : col0 + BLOCK].astype(
                    np.float32
                )
                rhs = xh[32 * b : 32 * b + 32, t.m, :].astype(np.float32)
            psum[key][32 * t.j : 32 * t.j + 32, :] += lhsT.T @ rhs
    for q in range(NWIN):
        acc = sum(psum[(q, i)] for i in range(4))
        o_d[q] = acc + bias_np[:, q : q + 1]
    return o_d


def kernel(x, weight_data, bias, block_ids):
    x = np.ascontiguousarray(np.asarray(x, dtype=np.float32))
    weight_data = np.asarray(weight_data, dtype=np.float32)
    bias = np.asarray(bias, dtype=np.float32)
    block_ids = np.asarray(block_ids)

    sched = _schedule(block_ids)
    w_np = _prep_weights(weight_data, sched)
    bias_np = _prep_bias(bias, sched)
    xhs = [_prep_x(x[c * TSH : (c + 1) * TSH]) for c in range(N_CORES)]

    if bool(int(os.environ.get("BSL_EMU", "0"))):
        o_cores = [_emulate_core(xh, w_np, bias_np, sched) for xh in xhs]
        return _assemble_out(o_cores, sched)

    in_maps = [{"xh": xhs[c], "wst": w_np, "biasq": bias_np} for c in range(N_CORES)]

    nc = _build_bass(sched)
    trace = bool(int(os.environ.get("BSL_TRACE", "0")))
    if trace:
        trace = _install_axon_ntff_hook()
    kwargs = {}
    if trace:
        tdir = os.environ.get("BSL_TRACE_DIR")
        if tdir:
            os.makedirs(tdir, exist_ok=True)
            kwargs["tmpdir"] = tdir
        kwargs["trace_cores"] = list(range(N_CORES))
    res = run_bass_kernel_spmd(
        nc,
        in_maps,
        core_ids=list(range(N_CORES)),
        trace=trace,
        **kwargs,
    )

    global LAST_EXEC_NS, LAST_RESULT
    LAST_EXEC_NS = res.exec_time_ns
    LAST_RESULT = res

    o_cores = [res.results[c]["out"] for c in range(N_CORES)]
    return _assemble_out(o_cores, sched)


# revision 9
# speedup vs baseline: 1.1713x; 1.1713x over previous
"""BlockSparseLinear on 8 TRN2 NeuronCores — block-sparse PE-tiled kernel (v3).

Computes out = x @ W_dense.T + bias where W_dense is [4096, 4096] assembled
from 8192 nonzero 32x32 blocks (50% density).

Strategy:
  - Pure 8-way token sharding: each core gets 512 tokens, all 8192 blocks.
    The sparsity pattern is shared, so all cores run one SPMD program.
  - Only nonzero blocks are computed, via PE sub-array tiling.  The v2
    lesson: every matmul pays ~34ns on the serial weight-load path
    (LDWEIGHTS streams its 32 columns at 1.2 GHz) regardless of stationary
    height.  So blocks are packed into the TALLEST stationaries possible:
    an x "slot" m holds 4 k-blocks (bands i=0..3 at partitions 32i); a row
    with >=2 blocks in one slot gets a single [128k, 32o] stationary
    (QUAD, absent bands zeroed, one LDW for up to 4 blocks).  Lone blocks
    stay [32k, 32o] SINGLEs.  A global knob balances total instruction
    issue (~34ns each) against PE stream occupancy (~15ns per block-slot).
  - Rows are processed in 32 windows of 4 rows (one per column strip j).
    PSUM discipline: tile -> bank 4p + lowest-band (p = window parity);
    same-(bank,slice) writers always share a sub-array path so hardware
    FIFO serializes them; different slices of one bank may be written
    concurrently (v2-proven).  Dummy zero-weight singles cover (row, band)
    groups that would otherwise leave a psum slice unstarted.
  - Per window, quads are emitted first, then singles (fewer PE tiling
    mode switches), each round-robin across strips/bands.
  - DVE combines the 4 partial banks + bias per window -> DMA out.
"""

import os
from itertools import permutations

import numpy as np
from ml_dtypes import bfloat16

import concourse.mybir as mybir
import concourse.tile as tile
from concourse import bacc
from concourse.bass_utils import run_bass_kernel_spmd

BLOCK = 32
IN_FEATURES = 4096
OUT_FEATURES = 4096
N_TOKENS = 4096
IB = IN_FEATURES // BLOCK  # 128 block-cols
OB = OUT_FEATURES // BLOCK  # 128 block-rows

N_CORES = 8
TSH = N_TOKENS // N_CORES  # 512 tokens per core
NFREE = 512
P = 128

NWIN = 32  # windows of 4 rows
CHUNK_SLOTS = 64  # 32-col weight slots per DMA chunk
X_CHUNKS = 8

ISSUE_NS = 34.0  # measured per-instruction issue cost (LDW+MM pair)
UNIT_NS = 15.0  # per block-slot stream cost (512 cyc / 16 sub-arrays)

LAST_EXEC_NS = None
LAST_RESULT = None


def _install_axon_ntff_hook():
    try:
        from antenv.axon_hooks import get_axon_ntff_profile_hook

        return get_axon_ntff_profile_hook() is not None
    except ImportError:
        pass
    try:
        import sys
        import types

        import antenv
        import trn_agent_boot.trn_boot as tb

        hook = tb._ntff_profile_via_ctypes("/opt/axon/libaxon_pjrt.so")
        if hook is None:
            return False
        mod = types.ModuleType("antenv.axon_hooks")
        mod._hook = hook
        mod.get_axon_ntff_profile_hook = lambda: mod._hook
        mod.set_axon_ntff_profile_hook = lambda h: setattr(mod, "_hook", h)
        sys.modules["antenv.axon_hooks"] = mod
        antenv.axon_hooks = mod

        import concourse.bass_utils as bu

        bu.upload_artifacts = lambda tmpdir: str(tmpdir)
        return True
    except Exception:
        return False


class _Tile:
    """One PE instruction: a QUAD ([128,32] stationary) or SINGLE ([32,32])."""

    __slots__ = ("kind", "row", "m", "blocks", "j", "w", "slot", "start", "stop")

    def __init__(self, kind, row, m, blocks):
        self.kind = kind  # 'q' or 's'
        self.row = row
        self.m = m
        self.blocks = blocks  # list of (band, bidx); bidx -1 = zero dummy
        self.j = -1
        self.w = -1
        self.slot = -1
        self.start = False
        self.stop = False

    @property
    def lowband(self):
        return 0 if self.kind == "q" else self.blocks[0][0]


def _schedule(block_ids):
    ids = np.asarray(block_ids, dtype=np.int64)
    r_all = ids // IB
    c_all = ids % IB

    # per-row: dict m -> list of (band, bidx)
    row_slots = [dict() for _ in range(OB)]
    for b in range(len(ids)):
        r = int(r_all[b])
        c = int(c_all[b])
        row_slots[r].setdefault(c // 4, []).append((c % 4, b))

    # Phase-serial cost model (measured): quad = 53.3ns flat (stream-bound,
    # 4-way strip concurrency), single = 29.5ns flat (LDW-issue-bound).
    # k>=2 slots always win as quads; k==1 stays a single.
    row_tiles = []
    for r in range(OB):
        tiles = []
        for m in sorted(row_slots[r]):
            blks = sorted(row_slots[r][m])
            if len(blks) == 1:
                for band, bidx in blks:
                    tiles.append(_Tile("s", r, m, [(band, bidx)]))
            else:
                tiles.append(_Tile("q", r, m, blks))
        # guarantee every band group has a writer (psum slice validity)
        havebands = set(t.lowband for t in tiles)
        if not any(t.kind == "q" for t in tiles) and 0 not in havebands:
            pass  # band 0 covered below by dummy single if needed
        for b in range(4):
            if b not in havebands:
                tiles.append(_Tile("s", r, 0, [(b, -1)]))
        row_tiles.append(tiles)

    units = np.array(
        [sum(4 if t.kind == "q" else 1 for t in tiles) for tiles in row_tiles]
    )

    # windows: snake by units, then per-window strip assignment balancing
    order = np.argsort(-units, kind="stable")
    win_rows = [order[4 * w : 4 * w + 4] for w in range(NWIN)]
    tot_u = np.zeros(4)
    tot_q = np.zeros(4)
    assign = np.zeros((NWIN, 4), dtype=np.int64)  # [w, j] -> row
    for w in range(NWIN):
        rows4 = win_rows[w]
        u4 = units[rows4].astype(float)
        q4 = np.array(
            [sum(1 for t in row_tiles[r] if t.kind == "q") for r in rows4],
            dtype=float,
        )
        best = None
        for perm in permutations(range(4)):
            tu = tot_u.copy()
            tq = tot_q.copy()
            for k in range(4):
                tu[perm[k]] += u4[k]
                tq[perm[k]] += q4[k]
            score = (tq.max() - tq.min(), tu.max() - tu.min())
            if best is None or score < best[0]:
                best = (score, perm)
        perm = best[1]
        for k in range(4):
            j = perm[k]
            r = int(rows4[k])
            assign[w, j] = r
            for t in row_tiles[r]:
                t.j = j
                t.w = w
            tot_u[j] += u4[k]
            tot_q[j] += q4[k]

    # emission order + slot assignment; windows are processed in PAIRS so
    # each quad->single tiling-mode switch covers two windows (8 psum banks)
    emit = []  # list over window-pairs of instruction lists
    slot_base = 0
    for wp in range(NWIN // 2):
        wl = []
        # 4 PHYSICAL strip queues (both windows of the pair concatenated):
        # round-robin must cycle hardware strips, not (window, strip) pairs,
        # or head-of-queue blocking kills quad concurrency.
        qlists = [[], [], [], []]
        slists = [[], [], [], []]
        for w in (2 * wp, 2 * wp + 1):
            for j in range(4):
                r = int(assign[w, j])
                strips = row_tiles[r]
                qlists[j].extend(t for t in strips if t.kind == "q")
                ss = [t for t in strips if t.kind == "s"]
                # band-interleave this strip's singles
                byband = [[], [], [], []]
                for t in ss:
                    byband[t.lowband].append(t)
                k = 0
                while any(byband):
                    b = k % 4
                    if byband[b]:
                        slists[j].append(byband[b].pop(0))
                    k += 1
        qi = [0] * 4
        while True:
            prog = False
            for j in range(4):
                if qi[j] < len(qlists[j]):
                    t = qlists[j][qi[j]]
                    t.slot = slot_base
                    slot_base += 1
                    wl.append(t)
                    qi[j] += 1
                    prog = True
            if not prog:
                break
        lane = [0, 0, 0, 0]  # per-band lane counter for singles slots
        si = [0] * 4
        sl = []
        while True:
            prog = False
            for j in range(4):
                if si[j] < len(slists[j]):
                    t = slists[j][si[j]]
                    b = t.lowband
                    t.slot = slot_base + lane[b]
                    lane[b] += 1
                    sl.append(t)
                    si[j] += 1
                    prog = True
            if not prog:
                break
        slot_base += max(lane) if any(lane) else 0
        wl.extend(sl)
        emit.append(wl)

    # start/stop flags per (row, lowband) group, in emission order
    groups = {}
    for wl in emit:
        for t in wl:
            groups.setdefault((t.row, t.lowband), []).append(t)
    for key, ts in groups.items():
        ts[0].start = True
        ts[-1].stop = True

    n_slots = slot_base
    nch = (n_slots + CHUNK_SLOTS - 1) // CHUNK_SLOTS
    return {
        "emit": emit,
        "assign": assign,
        "NCH": nch,
        "n_slots": n_slots,
        "n_instr": sum(len(wl) for wl in emit),
    }


def _build_bass(sched):
    nch = sched["NCH"]
    emit = sched["emit"]

    nc = bacc.Bacc(None, target_bir_lowering=False)

    x_d = nc.dram_tensor("xh", [P, 32, TSH], mybir.dt.bfloat16, kind="ExternalInput")
    w_d = nc.dram_tensor(
        "wst", [nch, P, CHUNK_SLOTS * BLOCK], mybir.dt.bfloat16, kind="ExternalInput"
    )
    b_d = nc.dram_tensor("biasq", [P, NWIN], mybir.dt.float32, kind="ExternalInput")
    o_d = nc.dram_tensor("out", [NWIN, P, TSH], mybir.dt.float32, kind="ExternalOutput")

    with tile.TileContext(nc) as tc:
        with (
            tc.tile_pool(name="xpool", bufs=1) as xpool,
            tc.tile_pool(name="wpool", bufs=4) as wpool,
            tc.tile_pool(name="spool", bufs=2) as spool,
            tc.tile_pool(name="bpool", bufs=1) as bpool,
            tc.tile_pool(name="psum", bufs=2, space="PSUM") as ppool,
        ):
            bias_sb = bpool.tile([P, NWIN], mybir.dt.float32)
            nc.scalar.dma_start(bias_sb[:], b_d[:])

            # x in per-chunk tiles so matmul deps are chunk-granular (the
            # first matmuls must not wait for the whole 4.2MB of x)
            MCH = 32 // X_CHUNKS
            x_tiles = []
            for xc in range(X_CHUNKS):
                xt = xpool.tile(
                    [P, MCH, TSH], mybir.dt.bfloat16, tag=f"x{xc}", name="x"
                )
                x_tiles.append(xt)

            w_tiles = {}
            W_PREFETCH = 4

            def issue_w(ch):
                if ch < nch and ch not in w_tiles:
                    w_sb = wpool.tile(
                        [P, CHUNK_SLOTS * BLOCK], mybir.dt.bfloat16, tag="w", name="w"
                    )
                    nc.sync.dma_start(w_sb[:], w_d[ch])
                    w_tiles[ch] = w_sb

            # interleave the x and early-w DMAs across the two rings:
            # scalar: x0, x2, x4, x6  |  sync: w0, x1, w1, x3, w2, x5, w3, x7
            nc.scalar.dma_start(x_tiles[0][:], x_d[:, 0:MCH, :])
            issue_w(0)
            for k, xc in enumerate((1, 3, 5, 7)):
                issue_w(k + 1)
                nc.sync.dma_start(
                    x_tiles[xc][:], x_d[:, xc * MCH : (xc + 1) * MCH, :]
                )
            for xc in (2, 4, 6):
                nc.scalar.dma_start(
                    x_tiles[xc][:], x_d[:, xc * MCH : (xc + 1) * MCH, :]
                )

            ps_tiles = {}
            win_left = [16] * NWIN
            issued_ch = 0

            for wl in emit:
                for t in wl:
                    ch = t.slot // CHUNK_SLOTS
                    while issued_ch < ch:
                        issued_ch += 1
                        issue_w(issued_ch + W_PREFETCH)
                    w_sb = w_tiles[ch]
                    col0 = (t.slot % CHUNK_SLOTS) * BLOCK
                    key = (t.w, t.lowband)
                    if key not in ps_tiles:
                        ps_tiles[key] = ppool.tile(
                            [P, NFREE],
                            mybir.dt.float32,
                            tag=f"ps{t.lowband}",
                            name="ps",
                        )
                    psum_t = ps_tiles[key]
                    j = t.j
                    x_sb = x_tiles[t.m // MCH]
                    mloc = t.m % MCH
                    if t.kind == "q":
                        nc.tensor.matmul(
                            psum_t[32 * j : 32 * j + 32, :],
                            lhsT=w_sb[:, col0 : col0 + BLOCK],
                            rhs=x_sb[:, mloc, :],
                            start=t.start,
                            stop=t.stop,
                            tile_position=(0, 32 * j),
                        )
                    else:
                        b = t.lowband
                        nc.tensor.matmul(
                            psum_t[32 * j : 32 * j + 32, :],
                            lhsT=w_sb[32 * b : 32 * b + 32, col0 : col0 + BLOCK],
                            rhs=x_sb[32 * b : 32 * b + 32, mloc, :],
                            start=t.start,
                            stop=t.stop,
                            tile_position=(32 * b, 32 * j),
                        )
                    if t.stop:
                        win_left[t.w] -= 1
                        if win_left[t.w] == 0:
                            q = t.w
                            pt = [ps_tiles.pop((q, ii)) for ii in range(4)]
                            s1 = spool.tile([P, NFREE], mybir.dt.float32, tag="s1")
                            nc.vector.tensor_tensor(
                                s1[:],
                                pt[0][:],
                                bias_sb[:, q : q + 1].to_broadcast([P, NFREE]),
                                mybir.AluOpType.add,
                            )
                            s2 = spool.tile([P, NFREE], mybir.dt.float32, tag="s2")
                            nc.vector.tensor_tensor(
                                s2[:], pt[1][:], s1[:], mybir.AluOpType.add
                            )
                            s3 = spool.tile([P, NFREE], mybir.dt.float32, tag="s3")
                            nc.vector.tensor_tensor(
                                s3[:], pt[2][:], s2[:], mybir.AluOpType.add
                            )
                            so = spool.tile([P, NFREE], mybir.dt.float32, tag="so")
                            nc.vector.tensor_tensor(
                                so[:], pt[3][:], s3[:], mybir.AluOpType.add
                            )
                            nc.scalar.dma_start(o_d[q], so[:])

    nc.compile()
    return nc


def _prep_weights(weight_data, sched):
    nch = sched["NCH"]
    wdT = np.ascontiguousarray(weight_data.transpose(0, 2, 1)).astype(bfloat16)
    w_np = np.zeros((nch, P, CHUNK_SLOTS * BLOCK), dtype=bfloat16)
    for wl in sched["emit"]:
        for t in wl:
            ch = t.slot // CHUNK_SLOTS
            col0 = (t.slot % CHUNK_SLOTS) * BLOCK
            for band, bidx in t.blocks:
                if bidx >= 0:
                    w_np[ch, 32 * band : 32 * band + 32, col0 : col0 + BLOCK] = wdT[
                        bidx
                    ]
    return w_np


def _prep_x(x_shard):
    a = np.ascontiguousarray(x_shard.T).reshape(IB, BLOCK, TSH)  # [c, q, t]
    b = a.reshape(32, 4, BLOCK, TSH).transpose(1, 2, 0, 3)  # [i, q, m, t]
    return np.ascontiguousarray(b.reshape(P, 32, TSH)).astype(bfloat16)


def _prep_bias(bias, sched):
    assign = sched["assign"]
    bias_np = np.zeros((P, NWIN), dtype=np.float32)
    for q in range(NWIN):
        for j in range(4):
            r = int(assign[q, j])
            bias_np[32 * j : 32 * j + 32, q] = bias[32 * r : 32 * r + 32]
    return bias_np


def _assemble_out(o_cores, sched):
    assign = sched["assign"]
    rflat = assign.reshape(-1)
    out = np.empty((N_TOKENS, OUT_FEATURES), dtype=np.float32)
    for core, o in enumerate(o_cores):
        o4 = o.reshape(NWIN, 4, BLOCK, TSH)
        flat = o4.transpose(3, 0, 1, 2).reshape(TSH, OB, BLOCK)
        view = out[core * TSH : (core + 1) * TSH].reshape(TSH, OB, BLOCK)
        view[:, rflat, :] = flat
    return out


def _emulate_core(xh, w_np, bias_np, sched):
    o_d = np.zeros((NWIN, P, TSH), dtype=np.float32)
    psum = {}
    for wl in sched["emit"]:
        for t in wl:
            key = (t.w, t.lowband)
            if key not in psum:
                psum[key] = np.zeros((P, NFREE), dtype=np.float32)
            if t.start:
                psum[key][32 * t.j : 32 * t.j + 32, :] = 0.0
            ch = t.slot // CHUNK_SLOTS
            col0 = (t.slot % CHUNK_SLOTS) * BLOCK
            if t.kind == "q":
                lhsT = w_np[ch, :, col0 : col0 + BLOCK].astype(np.float32)
                rhs = xh[:, t.m, :].astype(np.float32)
            else:
                b = t.lowband
                lhsT = w_np[ch, 32 * b : 32 * b + 32, col0 : col0 + BLOCK].astype(
                    np.float32
                )
                rhs = xh[32 * b : 32 * b + 32, t.m, :].astype(np.float32)
            psum[key][32 * t.j : 32 * t.j + 32, :] += lhsT.T @ rhs
    for q in range(NWIN):
        acc = sum(psum[(q, i)] for i in range(4))
        o_d[q] = acc + bias_np[:, q : q + 1]
    return o_d


def kernel(x, weight_data, bias, block_ids):
    x = np.ascontiguousarray(np.asarray(x, dtype=np.float32))
    weight_data = np.asarray(weight_data, dtype=np.float32)
    bias = np.asarray(bias, dtype=np.float32)
    block_ids = np.asarray(block_ids)

    sched = _schedule(block_ids)
    w_np = _prep_weights(weight_data, sched)
    bias_np = _prep_bias(bias, sched)
    xhs = [_prep_x(x[c * TSH : (c + 1) * TSH]) for c in range(N_CORES)]

    if bool(int(os.environ.get("BSL_EMU", "0"))):
        o_cores = [_emulate_core(xh, w_np, bias_np, sched) for xh in xhs]
        return _assemble_out(o_cores, sched)

    in_maps = [{"xh": xhs[c], "wst": w_np, "biasq": bias_np} for c in range(N_CORES)]

    nc = _build_bass(sched)
    trace = bool(int(os.environ.get("BSL_TRACE", "0")))
    if trace:
        trace = _install_axon_ntff_hook()
    kwargs = {}
    if trace:
        tdir = os.environ.get("BSL_TRACE_DIR")
        if tdir:
            os.makedirs(tdir, exist_ok=True)
            kwargs["tmpdir"] = tdir
        kwargs["trace_cores"] = list(range(N_CORES))
    res = run_bass_kernel_spmd(
        nc,
        in_maps,
        core_ids=list(range(N_CORES)),
        trace=trace,
        **kwargs,
    )

    global LAST_EXEC_NS, LAST_RESULT
    LAST_EXEC_NS = res.exec_time_ns
    LAST_RESULT = res

    o_cores = [res.results[c]["out"] for c in range(N_CORES)]
    return _assemble_out(o_cores, sched)
